# revision 26
# baseline (speedup 1.0000x reference)
"""Multi-head attention (B=2, S=2048, E=1024, H=16, D=64) on 8 TRN2 cores.

Sharding: core c handles batch b = c//4 and head-group g = c%4 (4 heads,
256 embed cols). No cross-core communication; host slices inputs (pre-
transposed and pre-cast to bf16) and gathers/normalizes outputs.

Per-core device program (fp16 matmuls, fp32 PSUM accumulation):
  - DMA is streamed in 512-column s-chunks in consumption-deadline order
    (k0 q0 k1 k2 k3 q1 v0 v1 v2 q2 v3 q3) so the first exp fires ~12us in
    instead of waiting for all 12MB. The host lays each chunk out as
    [sb][p][et][s'] so one chunk = 128 descriptors x 8KB (HWDGE descriptor
    generation at ~100/us is the limiter for small rows); weights are
    [p][et*cols] (4KB rows) with the q-bias folded into wq's last columns.
  - projections: qhT[c, s] = sum_e wq[e, c] qT[e, s] (c on partitions), so
    attention needs no on-chip transposes; K-bias dropped (softmax-invariant),
    V-bias applied on host (distributes through softmax). Projections are
    emitted per s-chunk: (ch0, sb0) of K and Q up front, everything else
    (K/Q ch0 rest, all ch1, all of V) as deadline-scheduled filler chunks
    inside the attention loop, using PE slack under the ScalarE exp cadence.
  - attention processes head PAIRS: the two scores matmuls run concurrently
    on disjoint PE row groups (K=64 each) into one [128, 1024] PSUM tile;
    one ScalarE exp (scale=1/8 fused) covers both heads per (iq, jt).
    ScalarE exp (~1.05us per [128,1024] tile x 128 steps) is the roofline;
    everything else hides under it.
  - vh carries a ones column (m=64), so the out-stage accumulates the
    softmax denominator in PSUM row 64; host divides. out-matmuls lag
    DEPTH steps behind exp so V projection has time; the lag drains at
    2 pairs/step over the last steps to shrink the tail.
"""

import sys

sys.path.insert(0, "/opt/trn_rl_repo")

import os

import numpy as np

if os.environ.get("JAX_PLATFORMS") == "cpu":
    # the bass program must run on the neuron cores; the axon/neuron PJRT
    # platform registers only when JAX_PLATFORMS is unset/empty
    del os.environ["JAX_PLATFORMS"]

import concourse.bass as bass  # noqa: F401
import concourse.mybir as mybir
from concourse import bacc
from concourse.tile import TileContext

B, S, E = 2, 2048, 1024
H, D = 16, 64
HPC = 4  # heads per core
COLS = HPC * D  # 256
P = 128
F32 = mybir.dt.float32
F16 = mybir.dt.float16
ET = E // P  # 8 e-tiles
JT = S // P  # 16 j-tiles
NB = 512
NIQ = S // NB  # 4 i-quarters
DEPTH = 22  # out-matmuls lag this many steps behind scores/exp

_CACHED = {}


def build():
    from collections import deque

    nc = bacc.Bacc("TRN2", target_bir_lowering=False, debug=False)
    # activations chunk-major: [sb, p, et, s'] so one s-chunk load is 128
    # descriptors of 8KB; wq carries bq in its last 2 columns
    qT = nc.dram_tensor("qT", [NIQ, P, ET, NB], F16, kind="ExternalInput")
    kT = nc.dram_tensor("kT", [NIQ, P, ET, NB], F16, kind="ExternalInput")
    vT = nc.dram_tensor("vT", [NIQ, P, ET, NB], F16, kind="ExternalInput")
    wq = nc.dram_tensor("wq", [P, ET * COLS + 2], F16, kind="ExternalInput")
    wk = nc.dram_tensor("wk", [P, ET * COLS], F16, kind="ExternalInput")
    wv = nc.dram_tensor("wv", [P, ET * COLS], F16, kind="ExternalInput")
    # out_raw[:, (h*NIQ+iq)*NB : ...]: rows 0-63 numerator (d), row 64 denom
    out_raw = nc.dram_tensor("out_raw", [65, HPC * S], F16,
                             kind="ExternalOutput")  # [65, 8192]

    with TileContext(nc) as tc:
        with (
            tc.tile_pool(name="wp", bufs=1) as wp,
            tc.tile_pool(name="xt", bufs=1) as xt,
            tc.tile_pool(name="hp", bufs=1) as hp,
            tc.tile_pool(name="pe", bufs=DEPTH + 6) as pe,
            tc.tile_pool(name="ob", bufs=4) as ob,
            tc.tile_pool(name="psA", bufs=2, space="PSUM") as psA,
            tc.tile_pool(name="psF", bufs=2, space="PSUM") as psF,
            tc.tile_pool(name="psOut", bufs=2, space="PSUM") as psOut,
        ):
            # --- weights + bias first (tiny; needed by every projection) ---
            wq_r = wp.tile([P, ET * COLS + 2], F16)
            wk_r = wp.tile([P, ET * COLS], F16)
            wv_r = wp.tile([P, ET * COLS], F16)
            nc.sync.dma_start(wq_r, wq[:, :])
            nc.sync.dma_start(wk_r, wk[:, :])
            nc.sync.dma_start(wv_r, wv[:, :])
            wq_b = wq_r[:, : ET * COLS].rearrange("p (t c) -> p t c", t=ET)
            wk_b = wk_r.rearrange("p (t c) -> p t c", t=ET)
            wv_b = wv_r.rearrange("p (t c) -> p t c", t=ET)
            bq_t = wp.tile([P, 2], F32)  # DVE bias scalars must be f32
            nc.vector.tensor_copy(bq_t, wq_r[:, ET * COLS : ET * COLS + 2])

            # --- activations as single [P, ET, S] tiles, loaded in 512-col
            # s-chunks in the order attention consumes them; sync HWDGE DMAs
            # execute FIFO so the transfer order matches issue order ---
            qx = xt.tile([P, NIQ, ET, NB], F16, name="qx")
            kx = xt.tile([P, NIQ, ET, NB], F16, name="kx")
            vx = xt.tile([P, NIQ, ET, NB], F16, name="vx")

            # the 8-deep DMA sem pool lets transfers run concurrently,
            # bandwidth-shared — which would delay the first chunks behind
            # later ones. Space the issues with tiny 1-descriptor dummy DMAs
            # (~0.6us issue cost each) so the 1MB chunks arrive ~serially,
            # in order, each at full bandwidth.
            pads = [
                wp.tile([1, NB], F16, tag=f"pad{i}", name=f"pad{i}")
                for i in range(4)
            ]

            def load_chunk(dst, dram, sb):
                nc.sync.dma_start(dst[:, sb], dram[sb])

            load_chunk(kx, kT, 0)
            for i in range(4):
                nc.sync.dma_start(pads[i], qT[0, 0:1, 0, :])
            load_chunk(qx, qT, 0)
            for dst, dram, sb in (
                (kx, kT, 1), (kx, kT, 2),
                (kx, kT, 3), (qx, qT, 1), (vx, vT, 0), (vx, vT, 1),
                (qx, qT, 2), (vx, vT, 2), (vx, vT, 3), (qx, qT, 3),
            ):
                for i in range(2):
                    nc.sync.dma_start(pads[i], qT[0, 0:1, 0, :])
                load_chunk(dst, dram, sb)

            # --- resident head tensors ---
            qhT = hp.tile([P, 2, S], F16)  # [2 heads x 64 d, chunk, s]
            khT = hp.tile([P, 2, S], F16)
            vh_aug = hp.tile([P, JT, HPC * 65], F16)
            # preload the exp table set (~2.7us) before the hot loop
            warm = pe.tile([P, 1024], F16, tag="e", name="warm")
            nc.scalar.activation(
                warm[:, 0:2], bq_t, mybir.ActivationFunctionType.Exp, scale=0.0
            )
            nc.vector.tensor_copy(vh_aug[:, 0, 0:2], warm[:, 0:2])
            nc.vector.memset(vh_aug, 1.0)

            # --- PE warmup: dummy matmuls on the weight tiles keep the PE
            # HAM busy through its cold window so real work runs at 2.4GHz ---
            psW = psF.tile([P, 256], F32, tag="f", name="warmmm")
            for i in range(10):
                nc.tensor.matmul(
                    psW, wq_b[:, i % ET, 0:P], wq_b[:, (i + 1) % ET, :],
                    start=True, stop=True,
                )

            # --- projection chunk generators (emitted incrementally) ---
            # one (ch, sb) K/Q chunk: 8 et-steps x 2 column-halves into 2 PSUM
            # slots so consecutive matmuls alternate banks and the (shared)
            # LDWEIGHTS pipelines instead of serializing.
            def qk_chunk(w_b, x, dst, bias, ch, sb):
                ph = [
                    psF.tile([P, 256], F32, tag="f", name=f"f{ch}{sb}{h}")
                    for h in range(2)
                ]
                for et in range(ET):
                    for h in range(2):
                        nc.tensor.matmul(
                            ph[h],
                            w_b[:, et, ch * P : (ch + 1) * P],
                            x[:, sb, et, h * 256 : (h + 1) * 256],
                            start=(et == 0),
                            stop=(et == ET - 1),
                        )
                    if et % 2 == 1 and et < ET - 1:
                        yield
                for h in range(2):
                    dsl = dst[:, ch, sb * NB + h * 256 : sb * NB + (h + 1) * 256]
                    if bias is not None:
                        nc.vector.tensor_scalar_add(dsl, ph[h], bias[:, ch : ch + 1])
                    else:
                        nc.vector.tensor_copy(dsl, ph[h])

            # two-sb K/Q chunk: full-width N=512 matmuls, half the MM count
            def qk_chunk2(w_b, x, dst, bias, ch, sb0):
                ph = [
                    psF.tile([P, NB], F32, tag="f", name=f"g{ch}{sb0}{j}")
                    for j in range(2)
                ]
                for et in range(ET):
                    for j in range(2):
                        nc.tensor.matmul(
                            ph[j],
                            w_b[:, et, ch * P : (ch + 1) * P],
                            x[:, sb0 + j, et, :],
                            start=(et == 0),
                            stop=(et == ET - 1),
                        )
                    if et % 2 == 1 and et < ET - 1:
                        yield
                for j in range(2):
                    dsl = dst[:, ch, (sb0 + j) * NB : (sb0 + j + 1) * NB]
                    if bias is not None:
                        nc.vector.tensor_scalar_add(dsl, ph[j], bias[:, ch : ch + 1])
                    else:
                        nc.vector.tensor_copy(dsl, ph[j])

            # one V chunk (two 128-col s-blocks): natural [s, c] layout
            def v_chunk(sc0):
                ph = [
                    psF.tile([P, COLS], F32, tag="f", name=f"v{sc0}{j}")
                    for j in range(2)
                ]
                for et in range(ET):
                    for j in range(2):
                        sc = sc0 + j
                        nc.tensor.matmul(
                            ph[j],
                            vx[:, sc // 4, et, (sc % 4) * P : (sc % 4 + 1) * P],
                            wv_b[:, et, :],
                            start=(et == 0),
                            stop=(et == ET - 1),
                        )
                    if et % 2 == 1 and et < ET - 1:
                        yield
                for j in range(2):
                    nc.vector.tensor_copy(
                        vh_aug[:, sc0 + j].rearrange("p (h x) -> p h x", x=65)[
                            :, :, :D
                        ],
                        ph[j].rearrange("p (h x) -> p h x", x=D),
                    )

            def run_all(gen):
                for _ in gen:
                    pass

            # --- up-front: (ch0, sb0) of K then Q — the first scores input ---
            run_all(qk_chunk(wk_b, kx, khT, None, 0, 0))
            run_all(qk_chunk(wq_b, qx, qhT, bq_t, 0, 0))

            # --- filler queue: [deadline_step, ready_step, generator] ---
            # deadlines from consumption (scores need khT/qhT chunk, out-mm
            # needs vh chunk at step jt+DEPTH, pr=1 needs ch1 from step 64);
            # ready_step estimates DMA arrival so emission order stays
            # realistic for the in-order engine FIFOs.
            fq = deque(
                [d, r, g]
                for d, r, g in (
                    (4, 2, qk_chunk(wk_b, kx, khT, None, 0, 1)),
                    (8, 5, qk_chunk(wk_b, kx, khT, None, 0, 2)),
                    (12, 9, qk_chunk(wk_b, kx, khT, None, 0, 3)),
                    (16, 12, qk_chunk(wq_b, qx, qhT, bq_t, 0, 1)),
                    (DEPTH + 0, 16, v_chunk(0)),
                    (DEPTH + 1, 16, v_chunk(2)),
                    (DEPTH + 3, 19, v_chunk(4)),
                    (DEPTH + 5, 19, v_chunk(6)),
                    (DEPTH + 7, 23, v_chunk(8)),
                    (DEPTH + 9, 23, v_chunk(10)),
                    (33, 20, qk_chunk(wq_b, qx, qhT, bq_t, 0, 2)),
                    (DEPTH + 12, 26, v_chunk(12)),
                    (DEPTH + 14, 26, v_chunk(14)),
                    (44, 2, qk_chunk2(wk_b, kx, khT, None, 1, 0)),
                    (48, 28, qk_chunk(wq_b, qx, qhT, bq_t, 0, 3)),
                    (52, 9, qk_chunk2(wk_b, kx, khT, None, 1, 2)),
                    (60, 11, qk_chunk2(wq_b, qx, qhT, bq_t, 1, 0)),
                    (90, 30, qk_chunk2(wq_b, qx, qhT, bq_t, 1, 2)),
                )
            )

            def pump(it):
                # advance the head filler: always when its deadline is near,
                # else at most one opportunistic sub-chunk per step once its
                # data should have arrived
                n = 0
                while fq:
                    d, r, g = fq[0]
                    urgent = (d - it) <= 4
                    if not urgent and (it < r or n >= 1):
                        break
                    try:
                        next(g)
                        n += 1
                    except StopIteration:
                        fq.popleft()
                        continue
                    if n >= 3:
                        break

            # --- attention, head pairs; software-pipelined emission ---
            steps = [(pr, iq, jt) for pr in range(2) for iq in range(NIQ)
                     for jt in range(JT)]
            ops = {}  # (pr, iq) -> (op0, op1)
            pending = deque()  # deferred (pr, iq, jt, expT)

            def emit_out(pr, iq, jt, expT):
                if jt == 0:
                    ops[(pr, iq)] = (
                        psOut.tile([P, NB], F32, tag="o", name="op0"),
                        psOut.tile([P, NB], F32, tag="o", name="op1"),
                    )
                op0, op1 = ops[(pr, iq)]
                for hh, op in ((0, op0), (1, op1)):
                    h = 2 * pr + hh
                    nc.tensor.matmul(
                        op[:65, :],
                        vh_aug[:, jt, h * 65 : (h + 1) * 65],
                        expT[:, hh * NB : (hh + 1) * NB],
                        start=(jt == 0),
                        stop=(jt == JT - 1),
                    )
                if jt == JT - 1:  # evacuate + store this iq's outputs
                    for hh, op in ((0, op0), (1, op1)):
                        r = (2 * pr + hh) * NIQ + iq
                        osb = ob.tile([P, NB], F16, tag="ob", name="osb")
                        nc.vector.tensor_copy(osb[:65, :], op[:65, :])
                        nc.sync.dma_start(
                            out_raw[:, r * NB : (r + 1) * NB], osb[:65, :]
                        )
                    del ops[(pr, iq)]

            nsteps = len(steps)
            for it, (pr, iq, jt) in enumerate(steps):
                sps = psA.tile([P, 1024], F32, tag="s", name="sps")
                for hh in range(2):  # row-group packed, concurrent
                    r0 = hh * D
                    nc.tensor.matmul(
                        sps[:, hh * NB : (hh + 1) * NB],
                        khT[r0 : r0 + D, pr, jt * P : (jt + 1) * P],
                        qhT[r0 : r0 + D, pr, iq * NB : (iq + 1) * NB],
                        start=True,
                        stop=True,
                    )
                expT = pe.tile([P, 1024], F16, tag="e", name="expT")
                nc.scalar.activation(
                    expT, sps, mybir.ActivationFunctionType.Exp, scale=0.125
                )
                pending.append((pr, iq, jt, expT))
                # steady-state lag DEPTH; ramp the lag down near the end so
                # the post-loop drain is short. fillers go LAST within the
                # step so a filler waiting on a PSUM slot or DMA can never
                # head-of-line-block the scores chain in the PE FIFO.
                cap = min(DEPTH, max(1, nsteps - 1 - it))
                while len(pending) > cap:
                    emit_out(*pending.popleft())
                pump(it)
            while fq:  # any unfinished fillers (shouldn't happen)
                try:
                    next(fq[0][2])
                except StopIteration:
                    fq.popleft()
            while pending:
                emit_out(*pending.popleft())
    nc.finalize()
    return nc


def _prep_in_maps(q, k, v, wq, bq, wk, bk, wv, bv):
    bf = np.float16
    q, k, v = (np.asarray(x, np.float32) for x in (q, k, v))
    wqb, wkb, wvb = (np.asarray(x, bf) for x in (wq, wk, wv))
    bq = np.asarray(bq, bf)

    def chunked(x):  # [S, E] -> [sb, p, et, s'] (e = et*128 + p)
        a = x.T.astype(bf).reshape(ET, P, NIQ, NB).transpose(2, 1, 0, 3)
        return np.ascontiguousarray(a)

    def wpack(w):  # [E, COLS] -> [p, et*COLS]
        a = w.reshape(ET, P, COLS).transpose(1, 0, 2).reshape(P, ET * COLS)
        return np.ascontiguousarray(a)

    qTc = [chunked(q[b]) for b in range(B)]
    kTc = [chunked(k[b]) for b in range(B)]
    vTc = [chunked(v[b]) for b in range(B)]
    in_maps = []
    for c in range(8):
        b, g = divmod(c, 4)
        cs = slice(g * COLS, (g + 1) * COLS)
        bq_pair = bq[cs].reshape(2, P).T  # [P, 2]
        in_maps.append(
            {
                "qT": qTc[b],
                "kT": kTc[b],
                "vT": vTc[b],
                "wq": np.ascontiguousarray(
                    np.concatenate([wpack(wqb[:, cs]), bq_pair], axis=1)
                ),
                "wk": wpack(wkb[:, cs]),
                "wv": wpack(wvb[:, cs]),
            }
        )
    return in_maps


def _make_runner(nc, n_cores=8):
    """Persistent jitted shard_map runner over the prebuilt Bass module."""
    import jax
    from jax.experimental.shard_map import shard_map
    from jax.sharding import Mesh, NamedSharding, PartitionSpec
    from concourse import bass2jax

    bass2jax.install_neuronx_cc_hook()

    in_names, out_names, out_avals, zero_outs = [], [], [], []
    for alloc in nc.m.functions[0].allocations:
        if not isinstance(alloc, mybir.MemoryLocationSet):
            continue
        name = alloc.memorylocations[0].name
        if alloc.kind == "ExternalInput":
            in_names.append(name)
        elif alloc.kind == "ExternalOutput":
            shape = tuple(alloc.tensor_shape)
            dtype = mybir.dt.np(alloc.dtype)
            out_avals.append(jax.core.ShapedArray(shape, dtype))
            zero_outs.append(np.zeros((n_cores * shape[0], *shape[1:]), dtype))
            out_names.append(name)
    pid_name = nc.partition_id_tensor.name if nc.partition_id_tensor else None
    if pid_name is not None:
        in_names = [n for n in in_names if n != pid_name]
    n_params = len(in_names)
    all_names = in_names + out_names + ([pid_name] if pid_name else [])

    def _body(*args):
        operands = list(args)
        if pid_name is not None:
            operands.append(bass2jax.partition_id_tensor())
        outs = bass2jax._bass_exec_p.bind(
            *operands,
            out_avals=tuple(out_avals),
            in_names=tuple(all_names),
            out_names=tuple(out_names),
            lowering_input_output_aliases=(),
            sim_require_finite=True,
            sim_require_nnan=True,
            nc=nc,
        )
        return tuple(outs)

    devices = jax.devices()[:n_cores]
    mesh = Mesh(np.asarray(devices), ("core",))
    nio = n_params + len(out_names)
    sharded = jax.jit(
        shard_map(
            _body,
            mesh=mesh,
            in_specs=(PartitionSpec("core"),) * nio,
            out_specs=(PartitionSpec("core"),) * len(out_names),
            check_rep=False,
        ),
        keep_unused=True,
    )
    row_sharding = NamedSharding(mesh, PartitionSpec("core"))
    zeros_dev = [jax.device_put(z, row_sharding) for z in zero_outs]

    def run(in_maps):
        concat_in = [
            np.concatenate([np.asarray(m[name]) for m in in_maps], axis=0)
            for name in in_names
        ]
        out_arrs = sharded(*concat_in, *zeros_dev)
        return [
            {
                name: np.asarray(out_arrs[i]).reshape(n_cores, *out_avals[i].shape)[c]
                for i, name in enumerate(out_names)
            }
            for c in range(n_cores)
        ]

    run.sharded = sharded
    run.in_names = in_names
    run.zeros_dev = zeros_dev
    run.row_sharding = row_sharding
    return run


def get_runner():
    if "run" not in _CACHED:
        _CACHED["nc"] = build()
        _CACHED["run"] = _make_runner(_CACHED["nc"])
    return _CACHED["run"]


def kernel(q, k, v, wq, bq, wk, bk, wv, bv):
    run = get_runner()
    in_maps = _prep_in_maps(q, k, v, wq, bq, wk, bk, wv, bv)
    results = run(in_maps)

    bv = np.asarray(bv, np.float32)
    out = np.empty((B, S, E), np.float32)
    for c in range(8):
        b, g = divmod(c, 4)
        raw = results[c]["out_raw"].astype(np.float32)  # [65, 8192]
        num = raw[:64].reshape(64, HPC, S)  # [d, h, i] (NIQ*NB = S)
        den = raw[64].reshape(HPC, S)
        for h in range(HPC):
            col0 = g * COLS + h * D
            o = num[:, h, :] / den[h][None, :]
            out[b, :, col0 : col0 + D] = o.T + bv[col0 : col0 + D][None, :]
    return out


# revision 29
# speedup vs baseline: 1.0115x; 1.0115x over previous
"""Multi-head attention (B=2, S=2048, E=1024, H=16, D=64) on 8 TRN2 cores.

Sharding: core c handles batch b = c//4 and head-group g = c%4 (4 heads,
256 embed cols). No cross-core communication; host slices inputs (pre-
transposed and pre-cast to bf16) and gathers/normalizes outputs.

Per-core device program (fp16 matmuls, fp32 PSUM accumulation):
  - DMA is streamed in 512-column s-chunks in consumption-deadline order
    (k0 q0 k1 k2 k3 q1 v0 v1 v2 q2 v3 q3) so the first exp fires ~12us in
    instead of waiting for all 12MB. The host lays each chunk out as
    [sb][p][et][s'] so one chunk = 128 descriptors x 8KB (HWDGE descriptor
    generation at ~100/us is the limiter for small rows); weights are
    [p][et*cols] (4KB rows) with the q-bias folded into wq's last columns.
  - projections: qhT[c, s] = sum_e wq[e, c] qT[e, s] (c on partitions), so
    attention needs no on-chip transposes; K-bias dropped (softmax-invariant),
    V-bias applied on host (distributes through softmax). Projections are
    emitted per s-chunk: (ch0, sb0) of K and Q up front, everything else
    (K/Q ch0 rest, all ch1, all of V) as deadline-scheduled filler chunks
    inside the attention loop, using PE slack under the ScalarE exp cadence.
  - attention processes head PAIRS: the two scores matmuls run concurrently
    on disjoint PE row groups (K=64 each) into one [128, 1024] PSUM tile;
    one ScalarE exp (scale=1/8 fused) covers both heads per (iq, jt).
    ScalarE exp (~1.05us per [128,1024] tile x 128 steps) is the roofline;
    everything else hides under it.
  - vh carries a ones column (m=64), so the out-stage accumulates the
    softmax denominator in PSUM row 64; host divides. out-matmuls lag
    DEPTH steps behind exp so V projection has time; the lag drains at
    2 pairs/step over the last steps to shrink the tail.
"""

import sys

sys.path.insert(0, "/opt/trn_rl_repo")

import os

import numpy as np

if os.environ.get("JAX_PLATFORMS") == "cpu":
    # the bass program must run on the neuron cores; the axon/neuron PJRT
    # platform registers only when JAX_PLATFORMS is unset/empty
    del os.environ["JAX_PLATFORMS"]

import concourse.bass as bass  # noqa: F401
import concourse.mybir as mybir
from concourse import bacc
from concourse.tile import TileContext

B, S, E = 2, 2048, 1024
H, D = 16, 64
HPC = 4  # heads per core
COLS = HPC * D  # 256
P = 128
F32 = mybir.dt.float32
F16 = mybir.dt.float16
ET = E // P  # 8 e-tiles
JT = S // P  # 16 j-tiles
NB = 512
NIQ = S // NB  # 4 i-quarters
DEPTH = 22  # out-matmuls lag this many steps behind scores/exp

_CACHED = {}


def build():
    from collections import deque

    nc = bacc.Bacc("TRN2", target_bir_lowering=False, debug=False)
    # activations chunk-major: [sb, p, et, s'] so one s-chunk load is 128
    # descriptors of 8KB; wq carries bq in its last 2 columns
    qT = nc.dram_tensor("qT", [NIQ, P, ET, NB], F16, kind="ExternalInput")
    kT = nc.dram_tensor("kT", [NIQ, P, ET, NB], F16, kind="ExternalInput")
    vT = nc.dram_tensor("vT", [NIQ, P, ET, NB], F16, kind="ExternalInput")
    wq = nc.dram_tensor("wq", [P, ET * COLS + 2], F16, kind="ExternalInput")
    wk = nc.dram_tensor("wk", [P, ET * COLS], F16, kind="ExternalInput")
    wv = nc.dram_tensor("wv", [P, ET * COLS], F16, kind="ExternalInput")
    # out_raw[:, (h*NIQ+iq)*NB : ...]: rows 0-63 numerator (d), row 64 denom
    out_raw = nc.dram_tensor("out_raw", [65, HPC * S], F16,
                             kind="ExternalOutput")  # [65, 8192]

    with TileContext(nc) as tc:
        with (
            tc.tile_pool(name="wp", bufs=1) as wp,
            tc.tile_pool(name="xt", bufs=1) as xt,
            tc.tile_pool(name="hp", bufs=1) as hp,
            tc.tile_pool(name="pe", bufs=DEPTH + 6) as pe,
            tc.tile_pool(name="ob", bufs=4) as ob,
            tc.tile_pool(name="psA", bufs=2, space="PSUM") as psA,
            tc.tile_pool(name="psF", bufs=2, space="PSUM") as psF,
            tc.tile_pool(name="psOut", bufs=2, space="PSUM") as psOut,
        ):
            # --- weights + bias first (tiny; needed by every projection) ---
            wq_r = wp.tile([P, ET * COLS + 2], F16)
            wk_r = wp.tile([P, ET * COLS], F16)
            wv_r = wp.tile([P, ET * COLS], F16)
            nc.sync.dma_start(wq_r, wq[:, :])
            nc.sync.dma_start(wk_r, wk[:, :])
            nc.sync.dma_start(wv_r, wv[:, :])
            wq_b = wq_r[:, : ET * COLS].rearrange("p (t c) -> p t c", t=ET)
            wk_b = wk_r.rearrange("p (t c) -> p t c", t=ET)
            wv_b = wv_r.rearrange("p (t c) -> p t c", t=ET)
            bq_t = wp.tile([P, 2], F32)  # DVE bias scalars must be f32
            nc.vector.tensor_copy(bq_t, wq_r[:, ET * COLS : ET * COLS + 2])

            # --- activations as single [P, ET, S] tiles, loaded in 512-col
            # s-chunks in the order attention consumes them; sync HWDGE DMAs
            # execute FIFO so the transfer order matches issue order ---
            qx = xt.tile([P, NIQ, ET, NB], F16, name="qx")
            kx = xt.tile([P, NIQ, ET, NB], F16, name="kx")
            vx = xt.tile([P, NIQ, ET, NB], F16, name="vx")

            # the 8-deep DMA sem pool lets transfers run concurrently,
            # bandwidth-shared — which would delay the first chunks behind
            # later ones. Space the issues with tiny 1-descriptor dummy DMAs
            # (~0.6us issue cost each) so the 1MB chunks arrive ~serially,
            # in order, each at full bandwidth.
            pads = [
                wp.tile([1, NB], F16, tag=f"pad{i}", name=f"pad{i}")
                for i in range(4)
            ]

            def load_chunk(dst, dram, sb):
                nc.sync.dma_start(dst[:, sb], dram[sb])

            load_chunk(kx, kT, 0)
            for i in range(2):
                nc.sync.dma_start(pads[i], qT[0, 0:1, 0, :])
            load_chunk(qx, qT, 0)
            for dst, dram, sb in (
                (kx, kT, 1), (kx, kT, 2),
                (kx, kT, 3), (qx, qT, 1), (vx, vT, 0), (vx, vT, 1),
                (qx, qT, 2), (vx, vT, 2), (vx, vT, 3), (qx, qT, 3),
            ):
                for i in range(2):
                    nc.sync.dma_start(pads[i], qT[0, 0:1, 0, :])
                load_chunk(dst, dram, sb)

            # --- resident head tensors ---
            qhT = hp.tile([P, 2, S], F16)  # [2 heads x 64 d, chunk, s]
            khT = hp.tile([P, 2, S], F16)
            vh_aug = hp.tile([P, JT, HPC * 65], F16)
            # preload the exp table set (~2.7us) before the hot loop
            warm = pe.tile([P, 1024], F16, tag="e", name="warm")
            nc.scalar.activation(
                warm[:, 0:2], bq_t, mybir.ActivationFunctionType.Exp, scale=0.0
            )
            nc.vector.tensor_copy(vh_aug[:, 0, 0:2], warm[:, 0:2])
            nc.vector.memset(vh_aug, 1.0)

            # --- PE warmup: dummy matmuls on the weight tiles keep the PE
            # HAM busy through its cold window so real work runs at 2.4GHz ---
            psW = psF.tile([P, 256], F32, tag="f", name="warmmm")
            for i in range(18):
                nc.tensor.matmul(
                    psW, wq_b[:, i % ET, 0:P], wq_b[:, (i + 1) % ET, :],
                    start=True, stop=True,
                )

            # --- projection chunk generators (emitted incrementally) ---
            # one (ch, sb) K/Q chunk: 8 et-steps x 2 column-halves into 2 PSUM
            # slots so consecutive matmuls alternate banks and the (shared)
            # LDWEIGHTS pipelines instead of serializing.
            def qk_chunk(w_b, x, dst, bias, ch, sb):
                ph = [
                    psF.tile([P, 256], F32, tag="f", name=f"f{ch}{sb}{h}")
                    for h in range(2)
                ]
                for et in range(ET):
                    for h in range(2):
                        nc.tensor.matmul(
                            ph[h],
                            w_b[:, et, ch * P : (ch + 1) * P],
                            x[:, sb, et, h * 256 : (h + 1) * 256],
                            start=(et == 0),
                            stop=(et == ET - 1),
                        )
                    if et % 2 == 1 and et < ET - 1:
                        yield
                for h in range(2):
                    dsl = dst[:, ch, sb * NB + h * 256 : sb * NB + (h + 1) * 256]
                    if bias is not None:
                        nc.vector.tensor_scalar_add(dsl, ph[h], bias[:, ch : ch + 1])
                    else:
                        nc.vector.tensor_copy(dsl, ph[h])

            # two-sb K/Q chunk: full-width N=512 matmuls, half the MM count
            def qk_chunk2(w_b, x, dst, bias, ch, sb0):
                ph = [
                    psF.tile([P, NB], F32, tag="f", name=f"g{ch}{sb0}{j}")
                    for j in range(2)
                ]
                for et in range(ET):
                    for j in range(2):
                        nc.tensor.matmul(
                            ph[j],
                            w_b[:, et, ch * P : (ch + 1) * P],
                            x[:, sb0 + j, et, :],
                            start=(et == 0),
                            stop=(et == ET - 1),
                        )
                    if et % 2 == 1 and et < ET - 1:
                        yield
                for j in range(2):
                    dsl = dst[:, ch, (sb0 + j) * NB : (sb0 + j + 1) * NB]
                    if bias is not None:
                        nc.vector.tensor_scalar_add(dsl, ph[j], bias[:, ch : ch + 1])
                    else:
                        nc.vector.tensor_copy(dsl, ph[j])

            # one V chunk (two 128-col s-blocks): natural [s, c] layout
            def v_chunk(sc0):
                ph = [
                    psF.tile([P, COLS], F32, tag="f", name=f"v{sc0}{j}")
                    for j in range(2)
                ]
                for et in range(ET):
                    for j in range(2):
                        sc = sc0 + j
                        nc.tensor.matmul(
                            ph[j],
                            vx[:, sc // 4, et, (sc % 4) * P : (sc % 4 + 1) * P],
                            wv_b[:, et, :],
                            start=(et == 0),
                            stop=(et == ET - 1),
                        )
                    if et % 2 == 1 and et < ET - 1:
                        yield
                for j in range(2):
                    nc.vector.tensor_copy(
                        vh_aug[:, sc0 + j].rearrange("p (h x) -> p h x", x=65)[
                            :, :, :D
                        ],
                        ph[j].rearrange("p (h x) -> p h x", x=D),
                    )

            def run_all(gen):
                for _ in gen:
                    pass

            # --- up-front: (ch0, sb0) of K then Q — the first scores input ---
            run_all(qk_chunk(wk_b, kx, khT, None, 0, 0))
            run_all(qk_chunk(wq_b, qx, qhT, bq_t, 0, 0))

            # --- filler queue: [deadline_step, ready_step, generator] ---
            # deadlines from consumption (scores need khT/qhT chunk, out-mm
            # needs vh chunk at step jt+DEPTH, pr=1 needs ch1 from step 64);
            # ready_step estimates DMA arrival so emission order stays
            # realistic for the in-order engine FIFOs.
            fq = deque(
                [d, r, g]
                for d, r, g in (
                    (4, 2, qk_chunk(wk_b, kx, khT, None, 0, 1)),
                    (8, 5, qk_chunk(wk_b, kx, khT, None, 0, 2)),
                    (12, 9, qk_chunk(wk_b, kx, khT, None, 0, 3)),
                    (16, 12, qk_chunk(wq_b, qx, qhT, bq_t, 0, 1)),
                    (DEPTH + 0, 16, v_chunk(0)),
                    (DEPTH + 1, 16, v_chunk(2)),
                    (DEPTH + 3, 19, v_chunk(4)),
                    (DEPTH + 5, 19, v_chunk(6)),
                    (DEPTH + 7, 23, v_chunk(8)),
                    (DEPTH + 9, 23, v_chunk(10)),
                    (33, 20, qk_chunk(wq_b, qx, qhT, bq_t, 0, 2)),
                    (DEPTH + 12, 26, v_chunk(12)),
                    (DEPTH + 14, 26, v_chunk(14)),
                    (44, 2, qk_chunk2(wk_b, kx, khT, None, 1, 0)),
                    (48, 28, qk_chunk(wq_b, qx, qhT, bq_t, 0, 3)),
                    (52, 9, qk_chunk2(wk_b, kx, khT, None, 1, 2)),
                    (60, 11, qk_chunk2(wq_b, qx, qhT, bq_t, 1, 0)),
                    (90, 30, qk_chunk2(wq_b, qx, qhT, bq_t, 1, 2)),
                )
            )

            def pump(it):
                # advance the head filler: always when its deadline is near,
                # else at most one opportunistic sub-chunk per step once its
                # data should have arrived
                n = 0
                while fq:
                    d, r, g = fq[0]
                    urgent = (d - it) <= 4
                    if not urgent and (it < r or n >= 1):
                        break
                    try:
                        next(g)
                        n += 1
                    except StopIteration:
                        fq.popleft()
                        continue
                    if n >= 3:
                        break

            # --- attention, head pairs; software-pipelined emission ---
            steps = [(pr, iq, jt) for pr in range(2) for iq in range(NIQ)
                     for jt in range(JT)]
            ops = {}  # (pr, iq) -> (op0, op1)
            pending = deque()  # deferred (pr, iq, jt, expT)

            def emit_out(pr, iq, jt, expT):
                if jt == 0:
                    ops[(pr, iq)] = (
                        psOut.tile([P, NB], F32, tag="o", name="op0"),
                        psOut.tile([P, NB], F32, tag="o", name="op1"),
                    )
                op0, op1 = ops[(pr, iq)]
                for hh, op in ((0, op0), (1, op1)):
                    h = 2 * pr + hh
                    nc.tensor.matmul(
                        op[:65, :],
                        vh_aug[:, jt, h * 65 : (h + 1) * 65],
                        expT[:, hh * NB : (hh + 1) * NB],
                        start=(jt == 0),
                        stop=(jt == JT - 1),
                    )
                if jt == JT - 1:  # evacuate + store this iq's outputs
                    for hh, op in ((0, op0), (1, op1)):
                        r = (2 * pr + hh) * NIQ + iq
                        osb = ob.tile([P, NB], F16, tag="ob", name="osb")
                        nc.vector.tensor_copy(osb[:65, :], op[:65, :])
                        nc.sync.dma_start(
                            out_raw[:, r * NB : (r + 1) * NB], osb[:65, :]
                        )
                    del ops[(pr, iq)]

            nsteps = len(steps)
            for it, (pr, iq, jt) in enumerate(steps):
                sps = psA.tile([P, 1024], F32, tag="s", name="sps")
                for hh in range(2):  # row-group packed, concurrent
                    r0 = hh * D
                    nc.tensor.matmul(
                        sps[:, hh * NB : (hh + 1) * NB],
                        khT[r0 : r0 + D, pr, jt * P : (jt + 1) * P],
                        qhT[r0 : r0 + D, pr, iq * NB : (iq + 1) * NB],
                        start=True,
                        stop=True,
                    )
                expT = pe.tile([P, 1024], F16, tag="e", name="expT")
                nc.scalar.activation(
                    expT, sps, mybir.ActivationFunctionType.Exp, scale=0.125
                )
                pending.append((pr, iq, jt, expT))
                # steady-state lag DEPTH; ramp the lag down near the end so
                # the post-loop drain is short. fillers go LAST within the
                # step so a filler waiting on a PSUM slot or DMA can never
                # head-of-line-block the scores chain in the PE FIFO.
                # +2 elasticity right after an iq-block boundary so the exp
                # chain doesn't stall on the previous block's psOut evac
                cap = min(DEPTH + (2 if jt < 3 else 0),
                          max(1, nsteps - 1 - it))
                while len(pending) > cap:
                    emit_out(*pending.popleft())
                pump(it)
            while fq:  # any unfinished fillers (shouldn't happen)
                try:
                    next(fq[0][2])
                except StopIteration:
                    fq.popleft()
            while pending:
                emit_out(*pending.popleft())
    nc.finalize()
    return nc


def _prep_in_maps(q, k, v, wq, bq, wk, bk, wv, bv):
    bf = np.float16
    q, k, v = (np.asarray(x, np.float32) for x in (q, k, v))
    wqb, wkb, wvb = (np.asarray(x, bf) for x in (wq, wk, wv))
    bq = np.asarray(bq, bf)

    def chunked(x):  # [S, E] -> [sb, p, et, s'] (e = et*128 + p)
        a = x.T.astype(bf).reshape(ET, P, NIQ, NB).transpose(2, 1, 0, 3)
        return np.ascontiguousarray(a)

    def wpack(w):  # [E, COLS] -> [p, et*COLS]
        a = w.reshape(ET, P, COLS).transpose(1, 0, 2).reshape(P, ET * COLS)
        return np.ascontiguousarray(a)

    qTc = [chunked(q[b]) for b in range(B)]
    kTc = [chunked(k[b]) for b in range(B)]
    vTc = [chunked(v[b]) for b in range(B)]
    in_maps = []
    for c in range(8):
        b, g = divmod(c, 4)
        cs = slice(g * COLS, (g + 1) * COLS)
        bq_pair = bq[cs].reshape(2, P).T  # [P, 2]
        in_maps.append(
            {
                "qT": qTc[b],
                "kT": kTc[b],
                "vT": vTc[b],
                "wq": np.ascontiguousarray(
                    np.concatenate([wpack(wqb[:, cs]), bq_pair], axis=1)
                ),
                "wk": wpack(wkb[:, cs]),
                "wv": wpack(wvb[:, cs]),
            }
        )
    return in_maps


def _make_runner(nc, n_cores=8):
    """Persistent jitted shard_map runner over the prebuilt Bass module."""
    import jax
    from jax.experimental.shard_map import shard_map
    from jax.sharding import Mesh, NamedSharding, PartitionSpec
    from concourse import bass2jax

    bass2jax.install_neuronx_cc_hook()

    in_names, out_names, out_avals, zero_outs = [], [], [], []
    for alloc in nc.m.functions[0].allocations:
        if not isinstance(alloc, mybir.MemoryLocationSet):
            continue
        name = alloc.memorylocations[0].name
        if alloc.kind == "ExternalInput":
            in_names.append(name)
        elif alloc.kind == "ExternalOutput":
            shape = tuple(alloc.tensor_shape)
            dtype = mybir.dt.np(alloc.dtype)
            out_avals.append(jax.core.ShapedArray(shape, dtype))
            zero_outs.append(np.zeros((n_cores * shape[0], *shape[1:]), dtype))
            out_names.append(name)
    pid_name = nc.partition_id_tensor.name if nc.partition_id_tensor else None
    if pid_name is not None:
        in_names = [n for n in in_names if n != pid_name]
    n_params = len(in_names)
    all_names = in_names + out_names + ([pid_name] if pid_name else [])

    def _body(*args):
        operands = list(args)
        if pid_name is not None:
            operands.append(bass2jax.partition_id_tensor())
        outs = bass2jax._bass_exec_p.bind(
            *operands,
            out_avals=tuple(out_avals),
            in_names=tuple(all_names),
            out_names=tuple(out_names),
            lowering_input_output_aliases=(),
            sim_require_finite=True,
            sim_require_nnan=True,
            nc=nc,
        )
        return tuple(outs)

    devices = jax.devices()[:n_cores]
    mesh = Mesh(np.asarray(devices), ("core",))
    nio = n_params + len(out_names)
    sharded = jax.jit(
        shard_map(
            _body,
            mesh=mesh,
            in_specs=(PartitionSpec("core"),) * nio,
            out_specs=(PartitionSpec("core"),) * len(out_names),
            check_rep=False,
        ),
        keep_unused=True,
    )
    row_sharding = NamedSharding(mesh, PartitionSpec("core"))
    zeros_dev = [jax.device_put(z, row_sharding) for z in zero_outs]

    def run(in_maps):
        concat_in = [
            np.concatenate([np.asarray(m[name]) for m in in_maps], axis=0)
            for name in in_names
        ]
        out_arrs = sharded(*concat_in, *zeros_dev)
        return [
            {
                name: np.asarray(out_arrs[i]).reshape(n_cores, *out_avals[i].shape)[c]
                for i, name in enumerate(out_names)
            }
            for c in range(n_cores)
        ]

    run.sharded = sharded
    run.in_names = in_names
    run.zeros_dev = zeros_dev
    run.row_sharding = row_sharding
    return run


def get_runner():
    if "run" not in _CACHED:
        _CACHED["nc"] = build()
        _CACHED["run"] = _make_runner(_CACHED["nc"])
    return _CACHED["run"]


def kernel(q, k, v, wq, bq, wk, bk, wv, bv):
    run = get_runner()
    in_maps = _prep_in_maps(q, k, v, wq, bq, wk, bk, wv, bv)
    results = run(in_maps)

    bv = np.asarray(bv, np.float32)
    out = np.empty((B, S, E), np.float32)
    for c in range(8):
        b, g = divmod(c, 4)
        raw = results[c]["out_raw"].astype(np.float32)  # [65, 8192]
        num = raw[:64].reshape(64, HPC, S)  # [d, h, i] (NIQ*NB = S)
        den = raw[64].reshape(HPC, S)
        for h in range(HPC):
            col0 = g * COLS + h * D
            o = num[:, h, :] / den[h][None, :]
            out[b, :, col0 : col0 + D] = o.T + bv[col0 : col0 + D][None, :]
    return out
